# revision 14
# baseline (speedup 1.0000x reference)
"""KAN EncoderNetwork kernel for 8 Trainium2 NeuronCores.

Strategy (data-parallel, batch sharded 8 ways, weights replicated):

Each KAN layer  out = silu(x) @ sb + einsum('big,iog->bo', B(x), coef*ss)
is reformulated as ONE matmul per layer over an expanded feature matrix:

  out^T[o,b] = sum_K W'[K,o] * F[K,b]

where for every 128-wide input chunk the feature rows are 8 spline basis
blocks + 1 silu block (9*din rows total).  The uniform-grid cubic
B-spline basis has the closed form (cardinal spline, t = 2.5x + 5.5):

  6*B_g(x) = relu(2-w)^3 - 4*relu(1-w)^3,   w = |2.5x + 3.5 - g|

computed on ScalarE (Abs/Relu) + custom VectorE ops, balanced across the
two engines.  Everything stays feature-major ([feat, batch]) so layer
outputs in PSUM feed the next layer's basis computation directly; only
the final layer is transposed back (TensorE) for the [batch, out] output.

Weights are pre-assembled host-side into bf16 W' matrices with rows
ordered (in_chunk, block g in 0..8, lane) matching the feature layout.
"""

import sys

sys.path.insert(0, "/opt/trn_rl_repo")

import numpy as np
import ml_dtypes

import concourse.bacc as bacc
import concourse.mybir as mybir
import concourse.tile as tile
from concourse.bass_utils import run_bass_kernel_spmd
from concourse.masks import make_identity
from concourse.dve_spec import Spec, Src0, Src1, C0, C1, C2, Zero, relu, sq, maxx, lower, _has_src1
from concourse.dve_uop import DveOpSpec
from concourse.dve_ops import (
    DveOp,
    OPS,
    _SUB_OPCODE_FOR_NAME,
    CUSTOM_DVE_SPECS,
    _CUSTOM_DVE_ROW_BASE,
)

F32 = mybir.dt.float32
BF16 = mybir.dt.bfloat16
AF = mybir.ActivationFunctionType

WIDTH = [512, 1024, 1024, 1024, 256]
NCORES = 8
BATCH = 4096
BPC = BATCH // NCORES  # 512 batch rows per core
NG = 8  # spline basis functions per input dim
NB = NG + 1  # feature blocks per 128-chunk (8 basis + 1 silu)

# which basis functions use the ACT pipeline (B: Abs+Relu on ScalarE then
# one VectorE poly op) vs the all-DVE pipeline (E: two 1-stream VectorE ops)
VARIANT_B = {3, 4, 5, 6, 7}


def _register_op(name, spec):
    if name in _SUB_OPCODE_FOR_NAME:
        for op in OPS:
            if op.name == name:
                return op
        raise RuntimeError(f"opcode row taken but op {name} missing")
    row = _CUSTOM_DVE_ROW_BASE + len(OPS)
    _SUB_OPCODE_FOR_NAME[name] = row
    shas = {}
    for ver in ("v3", "v4"):
        uops = lower(spec, ver=ver)
        shas[ver] = DveOpSpec(
            name=name, opcode=row, uops=uops, rd1_en=_has_src1(spec)
        ).sha(ver)
    op = DveOp(name, spec, subdim=False, uops_sha=shas)
    OPS.append(op)
    CUSTOM_DVE_SPECS[name] = spec
    return op


# q = relu(s0 - w)^3        (variant A, pass 1; 1 stream)
_a = relu(C0 - Src0)
KAN_CUBE_TENT = _register_op(
    "KAN_CUBE_TENT",
    Spec(
        body=sq(_a) * _a,
        reference=lambda in0, in1, s0, s1, imm2: np.maximum(s0 - in0, 0.0) ** 3,
    ),
)

# out = q + s1 * relu(s0 - w)^3   (variant A, pass 2; in0=q, in1=w; 2 streams)
_r = relu(C0 - Src1)
KAN_SPLINE_COMBINE = _register_op(
    "KAN_SPLINE_COMBINE",
    Spec(
        body=Src0 + sq(_r) * _r * C1,
        reference=lambda in0, in1, s0, s1, imm2: in0
        + s1 * np.maximum(s0 - in1, 0.0) ** 3,
    ),
)

# out = a^3 + s1 * relu(a - s0)^3   (in0 = a2 = relu(2-w); 1 stream)
_rb = relu(Src0 - C0)
KAN_TENT_POLY = _register_op(
    "KAN_TENT_POLY",
    Spec(
        body=sq(Src0) * Src0 + sq(_rb) * _rb * C1,
        reference=lambda in0, in1, s0, s1, imm2: in0**3
        + s1 * np.maximum(in0 - s0, 0.0) ** 3,
    ),
)

# a2 = relu(imm2 - |x*s0 + s1|)    (variant E pass 1; 1 stream, from x)
_u = Src0 * C0 + C1
_wabs = maxx(_u, Zero - _u)
KAN_A2_ABS = _register_op(
    "KAN_A2_ABS",
    Spec(
        body=relu(C2 - _wabs),
        reference=lambda in0, in1, s0, s1, imm2: np.maximum(
            imm2 - np.abs(in0 * s0 + s1), 0.0
        ),
    ),
)


def _chunk_groups(nic):
    """Basis-op batching: keep the first two chunks solo (short critical
    chain at layer boundaries), pair the rest."""
    groups = [[0]]
    if nic >= 2:
        groups.append([1])
    c = 2
    while c < nic:
        groups.append(list(range(c, min(c + 2, nic))))
        c += 2
    return groups


def _build_nc():
    nc = bacc.Bacc(trn_type="TRN2")
    xT_dr = nc.dram_tensor("xT", [WIDTH[0], BPC], F32, kind="ExternalInput")
    w_dr = [
        nc.dram_tensor(f"w{l}", [NB * WIDTH[l], WIDTH[l + 1]], BF16,
                       kind="ExternalInput")
        for l in range(4)
    ]
    out_dr = nc.dram_tensor("out", [BPC, WIDTH[4]], F32, kind="ExternalOutput")

    with tile.TileContext(nc) as tc:
        with (
            tc.tile_pool(name="const", bufs=1) as const_pool,
            tc.tile_pool(name="xt", bufs=2) as xt_pool,
            tc.tile_pool(name="ft", bufs=12) as ft_pool,
            tc.tile_pool(name="wt", bufs=8) as wt_pool,
            tc.tile_pool(name="tmp", bufs=4) as tmp_pool,
            tc.tile_pool(name="outp", bufs=1) as out_pool,
            tc.tile_pool(name="psum", bufs=8, space="PSUM") as psum_pool,
        ):
            # col g in 0..7: Abs bias 3.5-g ; col 8: +2.0 (variant-B Relu bias)
            bias = const_pool.tile([128, NB], F32, tag="bias")
            for g in range(NG):
                nc.gpsimd.memset(bias[:, g : g + 1], 3.5 - g)
            nc.gpsimd.memset(bias[:, NG : NG + 1], 2.0)
            ident = const_pool.tile([128, 128], F32, tag="ident")
            make_identity(nc, ident)

            nic0 = WIDTH[0] // 128
            xt = xt_pool.tile([128, nic0, BPC], F32, tag="xt")
            xT_r = xT_dr.rearrange("(c p) b -> p c b", p=128)
            # chunk 0 first, then the first weight tiles, then the rest
            nc.sync.dma_start(xt[:, 0:1, :], xT_r[:, 0:1, :])
            pre_wt = []
            for kb in range(3):
                wt = wt_pool.tile([128, WIDTH[1]], BF16, tag="wt",
                                  name=f"wt_pre_{kb}")
                nc.sync.dma_start(wt, w_dr[0][kb * 128 : (kb + 1) * 128, :])
                pre_wt.append(wt)
            for c in range(1, nic0):
                nc.sync.dma_start(xt[:, c : c + 1, :], xT_r[:, c : c + 1, :])

            def emit_fast_restart(l, src_psum):
                """First basis block of chunk 0 computed straight from the
                previous layer's PSUM so the PE restarts quickly."""
                a2 = tmp_pool.tile([128, BPC], F32, tag="qv",
                                   name=f"a2fr_{l}")
                nc.vector._custom_dve(KAN_A2_ABS, out=a2, in0=src_psum,
                                      s0=2.5, s1=3.5, imm2=2.0)
                ft0 = ft_pool.tile([128, NB, BPC], BF16, tag="ft",
                                   name=f"ft_{l}_0")
                nc.vector._custom_dve(KAN_TENT_POLY, out=ft0[:, 0, :],
                                      in0=a2, s0=1.0, s1=-4.0)
                return ft0

            def emit_copies(xt, src_psums, chunks):
                for i, c in enumerate(chunks):
                    if i % 2 == 0:
                        nc.scalar.copy(xt[:, c, :], src_psums[c])
                    else:
                        nc.vector.tensor_copy(xt[:, c, :], src_psums[c])

            def emit_basis(l, xt, c, ft, skip_g0=False):
                xa = xt[:, c, :]
                for g in range(NG):
                    if skip_g0 and g == 0:
                        continue
                    if g in VARIANT_B:
                        wv = tmp_pool.tile([128, BPC], F32, tag="wv",
                                           name=f"wv_{l}_{c}_{g}")
                        nc.scalar.activation(wv, xa, AF.Abs,
                                             bias=bias[:, g : g + 1],
                                             scale=2.5)
                        a2 = tmp_pool.tile([128, BPC], F32, tag="qv",
                                           name=f"a2_{l}_{c}_{g}")
                        nc.scalar.activation(a2, wv, AF.Relu,
                                             bias=bias[:, NG : NG + 1],
                                             scale=-1.0)
                    else:
                        a2 = tmp_pool.tile([128, BPC], F32, tag="qv",
                                           name=f"a2_{l}_{c}_{g}")
                        nc.vector._custom_dve(KAN_A2_ABS, out=a2, in0=xa,
                                              s0=2.5, s1=3.5 - g, imm2=2.0)
                    nc.vector._custom_dve(KAN_TENT_POLY, out=ft[:, g, :],
                                          in0=a2, s0=1.0, s1=-4.0)
                nc.scalar.activation(ft[:, NG, :], xa, AF.Silu)

            def emit_mms(l, c, ft, psums, ocs, col0, KB):
                dout = WIDTH[l + 1]
                ncol = len(ocs) * 128
                for g in range(NB):
                    kb = c * NB + g
                    if l == 0 and kb < len(pre_wt):
                        wt = pre_wt[kb]
                        wslice = lambda oc: wt[:, oc * 128 : (oc + 1) * 128]
                    else:
                        wt = wt_pool.tile([128, ncol], BF16, tag="wt",
                                          name=f"wt_{l}_{kb}_{col0}")
                        nc.sync.dma_start(
                            wt,
                            w_dr[l][kb * 128 : (kb + 1) * 128,
                                    col0 : col0 + ncol],
                        )
                        wslice = lambda oc: wt[:, (oc - ocs[0]) * 128 :
                                               (oc - ocs[0] + 1) * 128]
                    for oc in ocs:
                        nc.tensor.matmul(
                            psums[oc], wslice(oc), ft[:, g, :],
                            start=(kb == 0), stop=(kb == KB - 1),
                        )

            # ---- layers 0 and 1: plain per-chunk pipeline ----
            prev_psums = None
            for l in range(2):
                din, dout = WIDTH[l], WIDTH[l + 1]
                nic, noc = din // 128, dout // 128
                KB = NB * nic
                if prev_psums is not None:
                    xt = xt_pool.tile([128, nic, BPC], F32, tag="xt",
                                      name=f"xt_{l}")
                psums = [
                    psum_pool.tile([128, BPC], F32, tag="psum", name=f"ps_{l}_{i}")
                    for i in range(noc)
                ]
                for c in range(nic):
                    if c == 0 and prev_psums is not None:
                        ft = emit_fast_restart(l, prev_psums[0])
                        emit_copies(xt, prev_psums, range(nic))
                        emit_basis(l, xt, 0, ft, skip_g0=True)
                    else:
                        ft = ft_pool.tile([128, NB, BPC], BF16, tag="ft",
                                          name=f"ft_{l}_{c}")
                        emit_basis(l, xt, c, ft)
                    emit_mms(l, c, ft, psums, list(range(noc)), 0, KB)
                prev_psums = psums

            # ---- layer 2: out-chunk phase split (4 + 4 banks) so layer-3
            # basis production for early chunks overlaps phase B matmuls ----
            nic2, noc2 = WIDTH[2] // 128, WIDTH[3] // 128
            KB2 = NB * nic2
            xt2 = xt_pool.tile([128, nic2, BPC], F32, tag="xt", name="xt_2")
            psums2 = [
                psum_pool.tile([128, BPC], F32, tag="psum", name=f"ps_2_{i}")
                for i in range(noc2)
            ]
            l2_fts = []
            for c in range(nic2):
                if c == 0:
                    ft = emit_fast_restart(2, prev_psums[0])
                    emit_copies(xt2, prev_psums, range(nic2))
                    emit_basis(2, xt2, 0, ft, skip_g0=True)
                else:
                    ft = ft_pool.tile([128, NB, BPC], BF16, tag="ft",
                                      name=f"ft_2_{c}")
                    emit_basis(2, xt2, c, ft)
                l2_fts.append(ft)
                emit_mms(2, c, ft, psums2, [0, 1, 2, 3], 0, KB2)

            # between phases: layer-3 input chunks 0..3 + their basis
            nic3, noc3 = WIDTH[3] // 128, WIDTH[4] // 128
            KB3 = NB * nic3
            xt3 = xt_pool.tile([128, nic3, BPC], F32, tag="xt", name="xt_3")
            ft3_0 = emit_fast_restart(3, psums2[0])
            emit_copies(xt3, psums2, range(4))
            psums3 = [
                psum_pool.tile([128, BPC], F32, tag="psum", name=f"ps_3_{i}")
                for i in range(noc3)
            ]
            l3_fts = [ft3_0]
            emit_basis(3, xt3, 0, ft3_0, skip_g0=True)
            for c in range(1, 4):
                ft = ft_pool.tile([128, NB, BPC], BF16, tag="ft",
                                  name=f"ft_3_{c}")
                emit_basis(3, xt3, c, ft)
                l3_fts.append(ft)

            # layer-2 phase B — one out-chunk at a time, so each psum2[oc]
            # (= layer-3 input chunk oc) completes ~3/4 of a phase earlier
            # and its layer-3 basis production overlaps the remaining
            # phase-B matmuls instead of stacking up at the end
            for oc in range(4, 8):
                for c in range(nic2):
                    for g in range(NB):
                        kb = c * NB + g
                        wt = wt_pool.tile([128, 128], BF16, tag="wt",
                                          name=f"wtb_{kb}_{oc}")
                        nc.sync.dma_start(
                            wt,
                            w_dr[2][kb * 128 : (kb + 1) * 128,
                                    oc * 128 : (oc + 1) * 128],
                        )
                        nc.tensor.matmul(
                            psums2[oc], wt, l2_fts[c][:, g, :],
                            start=(kb == 0), stop=(kb == KB2 - 1),
                        )
                if oc % 2 == 0:
                    nc.scalar.copy(xt3[:, oc, :], psums2[oc])
                else:
                    nc.vector.tensor_copy(xt3[:, oc, :], psums2[oc])
                ft = ft_pool.tile([128, NB, BPC], BF16, tag="ft",
                                  name=f"ft_3_{oc}")
                emit_basis(3, xt3, oc, ft)
                l3_fts.append(ft)

            # layer-3 matmuls
            for c in range(nic3):
                emit_mms(3, c, l3_fts[c], psums3, list(range(noc3)), 0, KB3)

            # output: transpose back to [batch, out]
            s3 = out_pool.tile([128, noc3, BPC], F32, tag="s3")
            for oc in range(noc3):
                nc.scalar.copy(s3[:, oc, :], psums3[oc])
            outT = out_pool.tile([128, BPC // 128, WIDTH[4]], F32, tag="outT")
            out_r = out_dr.rearrange("(j p) o -> p j o", p=128)
            for j in range(BPC // 128):
                for oc in range(noc3):
                    pst = psum_pool.tile([128, 128], F32, tag="psum",
                                         name=f"pst_{j}_{oc}")
                    nc.tensor.transpose(
                        pst, s3[:, oc, j * 128 : (j + 1) * 128], ident
                    )
                    nc.vector.tensor_copy(
                        outT[:, j, oc * 128 : (oc + 1) * 128], pst
                    )
                nc.sync.dma_start(
                    out_r[:, j : j + 1, :], outT[:, j : j + 1, :]
                )
    nc.finalize()
    return nc


_NC_CACHE = []


def _get_nc():
    if not _NC_CACHE:
        _NC_CACHE.append(_build_nc())
    return _NC_CACHE[0]


def _build_weights(inp):
    ws = {}
    for l in range(4):
        din, dout = WIDTH[l], WIDTH[l + 1]
        coef = np.asarray(inp[f"coef{l}"], dtype=np.float32)
        sb = np.asarray(inp[f"sb{l}"], dtype=np.float32)
        ss = np.asarray(inp[f"ss{l}"], dtype=np.float32)
        spline_w = coef * ss[:, :, None] * (1.0 / 6.0)  # [din, dout, 8]
        nic = din // 128
        sp = spline_w.reshape(nic, 128, dout, NG).transpose(0, 3, 1, 2)
        base = sb.reshape(nic, 128, dout)[:, None]
        W = np.concatenate([sp, base], axis=1).reshape(nic * NB * 128, dout)
        ws[f"w{l}"] = np.ascontiguousarray(W).astype(ml_dtypes.bfloat16)
    return ws


def _run(inputs, trace=False, **kwargs):
    inp = {k: np.asarray(v) for k, v in inputs.items()}
    ws = _build_weights(inp)
    x = np.concatenate(
        [inp["inputs_y"].astype(np.float32), inp["inputs_u"].astype(np.float32)],
        axis=1,
    )
    xT = np.ascontiguousarray(x.T)  # [512 feat, 4096 batch]
    nc = _get_nc()
    in_maps = []
    for c in range(NCORES):
        m = {"xT": np.ascontiguousarray(xT[:, c * BPC : (c + 1) * BPC])}
        m.update(ws)
        in_maps.append(m)
    res = run_bass_kernel_spmd(
        nc, in_maps, core_ids=list(range(NCORES)), trace=trace, **kwargs
    )
    out = np.concatenate([r["out"] for r in res.results], axis=0)
    return out.astype(np.float32), res


def kernel(**inputs) -> np.ndarray:
    out, _ = _run(inputs)
    return out


# revision 15
# speedup vs baseline: 1.2330x; 1.2330x over previous
"""KAN EncoderNetwork kernel for 8 Trainium2 NeuronCores.

Strategy (data-parallel, batch sharded 8 ways, weights replicated):

Each KAN layer  out = silu(x) @ sb + einsum('big,iog->bo', B(x), coef*ss)
is reformulated as ONE matmul per layer over an expanded feature matrix:

  out^T[o,b] = sum_K W'[K,o] * F[K,b]

where for every 128-wide input chunk the feature rows are 8 spline basis
blocks + 1 silu block (9*din rows total).  The uniform-grid cubic
B-spline basis has the closed form (cardinal spline, t = 2.5x + 5.5):

  6*B_g(x) = relu(2-w)^3 - 4*relu(1-w)^3,   w = |2.5x + 3.5 - g|

computed on ScalarE (Abs/Relu) + custom VectorE ops, balanced across the
two engines.  Everything stays feature-major ([feat, batch]) so layer
outputs in PSUM feed the next layer's basis computation directly; only
the final layer is transposed back (TensorE) for the [batch, out] output.

Weights are pre-assembled host-side into bf16 W' matrices with rows
ordered (in_chunk, block g in 0..8, lane) matching the feature layout.
"""

import sys

sys.path.insert(0, "/opt/trn_rl_repo")

import numpy as np
import ml_dtypes

import concourse.bacc as bacc
import concourse.mybir as mybir
import concourse.tile as tile
from concourse.bass_utils import run_bass_kernel_spmd
from concourse.masks import make_identity
from concourse.dve_spec import Spec, Src0, Src1, C0, C1, C2, Zero, relu, sq, maxx, lower, _has_src1
from concourse.dve_uop import DveOpSpec
from concourse.dve_ops import (
    DveOp,
    OPS,
    _SUB_OPCODE_FOR_NAME,
    CUSTOM_DVE_SPECS,
    _CUSTOM_DVE_ROW_BASE,
)

F32 = mybir.dt.float32
BF16 = mybir.dt.bfloat16
AF = mybir.ActivationFunctionType

WIDTH = [512, 1024, 1024, 1024, 256]
NCORES = 8
BATCH = 4096
BPC = BATCH // NCORES  # 512 batch rows per core
NG = 8  # spline basis functions per input dim
NB = NG + 1  # feature blocks per 128-chunk (8 basis + 1 silu)

# which basis functions use the ACT pipeline (B: Abs+Relu on ScalarE then
# one VectorE poly op) vs the all-DVE pipeline (E: two 1-stream VectorE ops)
VARIANT_B = {3, 4, 5, 6, 7}


def _register_op(name, spec):
    if name in _SUB_OPCODE_FOR_NAME:
        for op in OPS:
            if op.name == name:
                return op
        raise RuntimeError(f"opcode row taken but op {name} missing")
    row = _CUSTOM_DVE_ROW_BASE + len(OPS)
    _SUB_OPCODE_FOR_NAME[name] = row
    shas = {}
    for ver in ("v3", "v4"):
        uops = lower(spec, ver=ver)
        shas[ver] = DveOpSpec(
            name=name, opcode=row, uops=uops, rd1_en=_has_src1(spec)
        ).sha(ver)
    op = DveOp(name, spec, subdim=False, uops_sha=shas)
    OPS.append(op)
    CUSTOM_DVE_SPECS[name] = spec
    return op


# q = relu(s0 - w)^3        (variant A, pass 1; 1 stream)
_a = relu(C0 - Src0)
KAN_CUBE_TENT = _register_op(
    "KAN_CUBE_TENT",
    Spec(
        body=sq(_a) * _a,
        reference=lambda in0, in1, s0, s1, imm2: np.maximum(s0 - in0, 0.0) ** 3,
    ),
)

# out = q + s1 * relu(s0 - w)^3   (variant A, pass 2; in0=q, in1=w; 2 streams)
_r = relu(C0 - Src1)
KAN_SPLINE_COMBINE = _register_op(
    "KAN_SPLINE_COMBINE",
    Spec(
        body=Src0 + sq(_r) * _r * C1,
        reference=lambda in0, in1, s0, s1, imm2: in0
        + s1 * np.maximum(s0 - in1, 0.0) ** 3,
    ),
)

# out = a^3 + s1 * relu(a - s0)^3   (in0 = a2 = relu(2-w); 1 stream)
_rb = relu(Src0 - C0)
KAN_TENT_POLY = _register_op(
    "KAN_TENT_POLY",
    Spec(
        body=sq(Src0) * Src0 + sq(_rb) * _rb * C1,
        reference=lambda in0, in1, s0, s1, imm2: in0**3
        + s1 * np.maximum(in0 - s0, 0.0) ** 3,
    ),
)

# a2 = relu(imm2 - |x*s0 + s1|)    (variant E pass 1; 1 stream, from x)
_u = Src0 * C0 + C1
_wabs = maxx(_u, Zero - _u)
KAN_A2_ABS = _register_op(
    "KAN_A2_ABS",
    Spec(
        body=relu(C2 - _wabs),
        reference=lambda in0, in1, s0, s1, imm2: np.maximum(
            imm2 - np.abs(in0 * s0 + s1), 0.0
        ),
    ),
)


def _chunk_groups(nic):
    """Basis-op batching: keep the first two chunks solo (short critical
    chain at layer boundaries), pair the rest."""
    groups = [[0]]
    if nic >= 2:
        groups.append([1])
    c = 2
    while c < nic:
        groups.append(list(range(c, min(c + 2, nic))))
        c += 2
    return groups


def _build_nc():
    nc = bacc.Bacc(trn_type="TRN2")
    xT_dr = nc.dram_tensor("xT", [WIDTH[0], BPC], F32, kind="ExternalInput")
    w_dr = [
        nc.dram_tensor(f"w{l}", [NB * WIDTH[l], WIDTH[l + 1]], BF16,
                       kind="ExternalInput")
        for l in range(4)
    ]
    out_dr = nc.dram_tensor("out", [BPC, WIDTH[4]], F32, kind="ExternalOutput")

    with tile.TileContext(nc) as tc:
        with (
            tc.tile_pool(name="const", bufs=1) as const_pool,
            tc.tile_pool(name="xt", bufs=2) as xt_pool,
            tc.tile_pool(name="ft", bufs=12) as ft_pool,
            tc.tile_pool(name="wt", bufs=8) as wt_pool,
            tc.tile_pool(name="tmp", bufs=4) as tmp_pool,
            tc.tile_pool(name="outp", bufs=1) as out_pool,
            tc.tile_pool(name="psum", bufs=8, space="PSUM") as psum_pool,
        ):
            # col g in 0..7: Abs bias 3.5-g ; col 8: +2.0 (variant-B Relu bias)
            bias = const_pool.tile([128, NB], F32, tag="bias")
            for g in range(NG):
                nc.gpsimd.memset(bias[:, g : g + 1], 3.5 - g)
            nc.gpsimd.memset(bias[:, NG : NG + 1], 2.0)
            ident = const_pool.tile([128, 128], F32, tag="ident")
            make_identity(nc, ident)

            nic0 = WIDTH[0] // 128
            xt = xt_pool.tile([128, nic0, BPC], F32, tag="xt")
            xT_r = xT_dr.rearrange("(c p) b -> p c b", p=128)
            # chunk 0 first, then the first weight tiles, then the rest
            nc.sync.dma_start(xt[:, 0:1, :], xT_r[:, 0:1, :])
            pre_wt = []
            for kb in range(3):
                wt = wt_pool.tile([128, WIDTH[1]], BF16, tag="wt",
                                  name=f"wt_pre_{kb}")
                nc.sync.dma_start(wt, w_dr[0][kb * 128 : (kb + 1) * 128, :])
                pre_wt.append(wt)
            for c in range(1, nic0):
                nc.sync.dma_start(xt[:, c : c + 1, :], xT_r[:, c : c + 1, :])

            def emit_fast_restart(l, src_psum):
                """First basis block of chunk 0 computed straight from the
                previous layer's PSUM so the PE restarts quickly."""
                a2 = tmp_pool.tile([128, BPC], F32, tag="qv",
                                   name=f"a2fr_{l}")
                nc.vector._custom_dve(KAN_A2_ABS, out=a2, in0=src_psum,
                                      s0=2.5, s1=3.5, imm2=2.0)
                ft0 = ft_pool.tile([128, NB, BPC], BF16, tag="ft",
                                   name=f"ft_{l}_0")
                nc.vector._custom_dve(KAN_TENT_POLY, out=ft0[:, 0, :],
                                      in0=a2, s0=1.0, s1=-4.0)
                return ft0

            def emit_copies(xt, src_psums, chunks):
                for i, c in enumerate(chunks):
                    if i % 2 == 0:
                        nc.scalar.copy(xt[:, c, :], src_psums[c])
                    else:
                        nc.vector.tensor_copy(xt[:, c, :], src_psums[c])

            def emit_basis(l, xt, c, ft, skip_g0=False):
                xa = xt[:, c, :]
                for g in range(NG):
                    if skip_g0 and g == 0:
                        continue
                    if g in VARIANT_B:
                        wv = tmp_pool.tile([128, BPC], F32, tag="wv",
                                           name=f"wv_{l}_{c}_{g}")
                        nc.scalar.activation(wv, xa, AF.Abs,
                                             bias=bias[:, g : g + 1],
                                             scale=2.5)
                        a2 = tmp_pool.tile([128, BPC], F32, tag="qv",
                                           name=f"a2_{l}_{c}_{g}")
                        nc.scalar.activation(a2, wv, AF.Relu,
                                             bias=bias[:, NG : NG + 1],
                                             scale=-1.0)
                    else:
                        a2 = tmp_pool.tile([128, BPC], F32, tag="qv",
                                           name=f"a2_{l}_{c}_{g}")
                        nc.vector._custom_dve(KAN_A2_ABS, out=a2, in0=xa,
                                              s0=2.5, s1=3.5 - g, imm2=2.0)
                    nc.vector._custom_dve(KAN_TENT_POLY, out=ft[:, g, :],
                                          in0=a2, s0=1.0, s1=-4.0)
                nc.scalar.activation(ft[:, NG, :], xa, AF.Silu)

            def emit_mms(l, c, ft, psums, ocs, col0, KB):
                dout = WIDTH[l + 1]
                ncol = len(ocs) * 128
                for g in range(NB):
                    kb = c * NB + g
                    if l == 0 and kb < len(pre_wt):
                        wt = pre_wt[kb]
                        wslice = lambda oc: wt[:, oc * 128 : (oc + 1) * 128]
                    else:
                        wt = wt_pool.tile([128, ncol], BF16, tag="wt",
                                          name=f"wt_{l}_{kb}_{col0}")
                        nc.sync.dma_start(
                            wt,
                            w_dr[l][kb * 128 : (kb + 1) * 128,
                                    col0 : col0 + ncol],
                        )
                        wslice = lambda oc: wt[:, (oc - ocs[0]) * 128 :
                                               (oc - ocs[0] + 1) * 128]
                    for oc in ocs:
                        nc.tensor.matmul(
                            psums[oc], wslice(oc), ft[:, g, :],
                            start=(kb == 0), stop=(kb == KB - 1),
                        )

            # ---- layers 0 and 1: plain per-chunk pipeline ----
            prev_psums = None
            for l in range(2):
                din, dout = WIDTH[l], WIDTH[l + 1]
                nic, noc = din // 128, dout // 128
                KB = NB * nic
                if prev_psums is not None:
                    xt = xt_pool.tile([128, nic, BPC], F32, tag="xt",
                                      name=f"xt_{l}")
                psums = [
                    psum_pool.tile([128, BPC], F32, tag="psum", name=f"ps_{l}_{i}")
                    for i in range(noc)
                ]
                for c in range(nic):
                    if c == 0 and prev_psums is not None:
                        ft = emit_fast_restart(l, prev_psums[0])
                        emit_copies(xt, prev_psums, range(nic))
                        emit_basis(l, xt, 0, ft, skip_g0=True)
                    else:
                        ft = ft_pool.tile([128, NB, BPC], BF16, tag="ft",
                                          name=f"ft_{l}_{c}")
                        emit_basis(l, xt, c, ft)
                    emit_mms(l, c, ft, psums, list(range(noc)), 0, KB)
                prev_psums = psums

            # ---- layer 2: out-chunk phase split (4 + 4 banks) so layer-3
            # basis production for early chunks overlaps phase B matmuls ----
            nic2, noc2 = WIDTH[2] // 128, WIDTH[3] // 128
            KB2 = NB * nic2
            xt2 = xt_pool.tile([128, nic2, BPC], F32, tag="xt", name="xt_2")
            psums2 = [
                psum_pool.tile([128, BPC], F32, tag="psum", name=f"ps_2_{i}")
                for i in range(noc2)
            ]
            l2_fts = []
            for c in range(nic2):
                if c == 0:
                    ft = emit_fast_restart(2, prev_psums[0])
                    emit_copies(xt2, prev_psums, range(nic2))
                    emit_basis(2, xt2, 0, ft, skip_g0=True)
                else:
                    ft = ft_pool.tile([128, NB, BPC], BF16, tag="ft",
                                      name=f"ft_2_{c}")
                    emit_basis(2, xt2, c, ft)
                l2_fts.append(ft)
                emit_mms(2, c, ft, psums2, [0, 1, 2, 3], 0, KB2)

            # between phases: layer-3 input chunks 0..3 + their basis
            nic3, noc3 = WIDTH[3] // 128, WIDTH[4] // 128
            KB3 = NB * nic3
            xt3 = xt_pool.tile([128, nic3, BPC], F32, tag="xt", name="xt_3")
            ft3_0 = emit_fast_restart(3, psums2[0])
            emit_copies(xt3, psums2, range(4))
            psums3 = [
                psum_pool.tile([128, BPC], F32, tag="psum", name=f"ps_3_{i}")
                for i in range(noc3)
            ]
            l3_fts = [ft3_0]
            emit_basis(3, xt3, 0, ft3_0, skip_g0=True)
            for c in range(1, 4):
                ft = ft_pool.tile([128, NB, BPC], BF16, tag="ft",
                                  name=f"ft_3_{c}")
                emit_basis(3, xt3, c, ft)
                l3_fts.append(ft)

            # layer-2 phase B — one out-chunk at a time, so each psum2[oc]
            # (= layer-3 input chunk oc) completes ~3/4 of a phase earlier
            # and its layer-3 basis production overlaps the remaining
            # phase-B matmuls instead of stacking up at the end
            for oa in (4, 6):
                for c in range(nic2):
                    for g in range(NB):
                        kb = c * NB + g
                        wt = wt_pool.tile([128, 256], BF16, tag="wt",
                                          name=f"wtb_{kb}_{oa}")
                        nc.sync.dma_start(
                            wt,
                            w_dr[2][kb * 128 : (kb + 1) * 128,
                                    oa * 128 : (oa + 2) * 128],
                        )
                        nc.tensor.matmul(
                            psums2[oa], wt[:, 0:128], l2_fts[c][:, g, :],
                            start=(kb == 0), stop=(kb == KB2 - 1),
                        )
                        nc.tensor.matmul(
                            psums2[oa + 1], wt[:, 128:256],
                            l2_fts[c][:, g, :],
                            start=(kb == 0), stop=(kb == KB2 - 1),
                        )
                for oc in (oa, oa + 1):
                    if oc % 2 == 0:
                        nc.scalar.copy(xt3[:, oc, :], psums2[oc])
                    else:
                        nc.vector.tensor_copy(xt3[:, oc, :], psums2[oc])
                    ft = ft_pool.tile([128, NB, BPC], BF16, tag="ft",
                                      name=f"ft_3_{oc}")
                    emit_basis(3, xt3, oc, ft)
                    l3_fts.append(ft)

            # layer-3 matmuls
            for c in range(nic3):
                emit_mms(3, c, l3_fts[c], psums3, list(range(noc3)), 0, KB3)

            # output: transpose back to [batch, out]
            s3 = out_pool.tile([128, noc3, BPC], F32, tag="s3")
            for oc in range(noc3):
                nc.scalar.copy(s3[:, oc, :], psums3[oc])
            outT = out_pool.tile([128, BPC // 128, WIDTH[4]], F32, tag="outT")
            out_r = out_dr.rearrange("(j p) o -> p j o", p=128)
            for j in range(BPC // 128):
                for oc in range(noc3):
                    pst = psum_pool.tile([128, 128], F32, tag="psum",
                                         name=f"pst_{j}_{oc}")
                    nc.tensor.transpose(
                        pst, s3[:, oc, j * 128 : (j + 1) * 128], ident
                    )
                    nc.vector.tensor_copy(
                        outT[:, j, oc * 128 : (oc + 1) * 128], pst
                    )
                nc.sync.dma_start(
                    out_r[:, j : j + 1, :], outT[:, j : j + 1, :]
                )
    nc.finalize()
    return nc


_NC_CACHE = []


def _get_nc():
    if not _NC_CACHE:
        _NC_CACHE.append(_build_nc())
    return _NC_CACHE[0]


def _build_weights(inp):
    ws = {}
    for l in range(4):
        din, dout = WIDTH[l], WIDTH[l + 1]
        coef = np.asarray(inp[f"coef{l}"], dtype=np.float32)
        sb = np.asarray(inp[f"sb{l}"], dtype=np.float32)
        ss = np.asarray(inp[f"ss{l}"], dtype=np.float32)
        spline_w = coef * ss[:, :, None] * (1.0 / 6.0)  # [din, dout, 8]
        nic = din // 128
        sp = spline_w.reshape(nic, 128, dout, NG).transpose(0, 3, 1, 2)
        base = sb.reshape(nic, 128, dout)[:, None]
        W = np.concatenate([sp, base], axis=1).reshape(nic * NB * 128, dout)
        ws[f"w{l}"] = np.ascontiguousarray(W).astype(ml_dtypes.bfloat16)
    return ws


def _run(inputs, trace=False, **kwargs):
    inp = {k: np.asarray(v) for k, v in inputs.items()}
    ws = _build_weights(inp)
    x = np.concatenate(
        [inp["inputs_y"].astype(np.float32), inp["inputs_u"].astype(np.float32)],
        axis=1,
    )
    xT = np.ascontiguousarray(x.T)  # [512 feat, 4096 batch]
    nc = _get_nc()
    in_maps = []
    for c in range(NCORES):
        m = {"xT": np.ascontiguousarray(xT[:, c * BPC : (c + 1) * BPC])}
        m.update(ws)
        in_maps.append(m)
    res = run_bass_kernel_spmd(
        nc, in_maps, core_ids=list(range(NCORES)), trace=trace, **kwargs
    )
    out = np.concatenate([r["out"] for r in res.results], axis=0)
    return out.astype(np.float32), res


def kernel(**inputs) -> np.ndarray:
    out, _ = _run(inputs)
    return out


# revision 16
# speedup vs baseline: 1.3376x; 1.0848x over previous
"""KAN EncoderNetwork kernel for 8 Trainium2 NeuronCores.

Strategy (data-parallel, batch sharded 8 ways, weights replicated):

Each KAN layer  out = silu(x) @ sb + einsum('big,iog->bo', B(x), coef*ss)
is reformulated as ONE matmul per layer over an expanded feature matrix:

  out^T[o,b] = sum_K W'[K,o] * F[K,b]

where for every 128-wide input chunk the feature rows are 8 spline basis
blocks + 1 silu block (9*din rows total).  The uniform-grid cubic
B-spline basis has the closed form (cardinal spline, t = 2.5x + 5.5):

  6*B_g(x) = relu(2-w)^3 - 4*relu(1-w)^3,   w = |2.5x + 3.5 - g|

computed on ScalarE (Abs/Relu) + custom VectorE ops, balanced across the
two engines.  Everything stays feature-major ([feat, batch]) so layer
outputs in PSUM feed the next layer's basis computation directly; only
the final layer is transposed back (TensorE) for the [batch, out] output.

Weights are pre-assembled host-side into bf16 W' matrices with rows
ordered (in_chunk, block g in 0..8, lane) matching the feature layout.
"""

import sys

sys.path.insert(0, "/opt/trn_rl_repo")

import numpy as np
import ml_dtypes

import concourse.bacc as bacc
import concourse.mybir as mybir
import concourse.tile as tile
from concourse.bass_utils import run_bass_kernel_spmd
from concourse.masks import make_identity
from concourse.dve_spec import Spec, Src0, Src1, C0, C1, C2, Zero, relu, sq, maxx, lower, _has_src1
from concourse.dve_uop import DveOpSpec
from concourse.dve_ops import (
    DveOp,
    OPS,
    _SUB_OPCODE_FOR_NAME,
    CUSTOM_DVE_SPECS,
    _CUSTOM_DVE_ROW_BASE,
)

F32 = mybir.dt.float32
BF16 = mybir.dt.bfloat16
AF = mybir.ActivationFunctionType

WIDTH = [512, 1024, 1024, 1024, 256]
NCORES = 8
BATCH = 4096
BPC = BATCH // NCORES  # 512 batch rows per core
NG = 8  # spline basis functions per input dim
NB = NG + 1  # feature blocks per 128-chunk (8 basis + 1 silu)

# which basis functions use the ACT pipeline (B: Abs+Relu on ScalarE then
# one VectorE poly op) vs the all-DVE pipeline (E: two 1-stream VectorE ops)
VARIANT_B = {3, 4, 5, 6, 7}


def _register_op(name, spec):
    if name in _SUB_OPCODE_FOR_NAME:
        for op in OPS:
            if op.name == name:
                return op
        raise RuntimeError(f"opcode row taken but op {name} missing")
    row = _CUSTOM_DVE_ROW_BASE + len(OPS)
    _SUB_OPCODE_FOR_NAME[name] = row
    shas = {}
    for ver in ("v3", "v4"):
        uops = lower(spec, ver=ver)
        shas[ver] = DveOpSpec(
            name=name, opcode=row, uops=uops, rd1_en=_has_src1(spec)
        ).sha(ver)
    op = DveOp(name, spec, subdim=False, uops_sha=shas)
    OPS.append(op)
    CUSTOM_DVE_SPECS[name] = spec
    return op


# q = relu(s0 - w)^3        (variant A, pass 1; 1 stream)
_a = relu(C0 - Src0)
KAN_CUBE_TENT = _register_op(
    "KAN_CUBE_TENT",
    Spec(
        body=sq(_a) * _a,
        reference=lambda in0, in1, s0, s1, imm2: np.maximum(s0 - in0, 0.0) ** 3,
    ),
)

# out = q + s1 * relu(s0 - w)^3   (variant A, pass 2; in0=q, in1=w; 2 streams)
_r = relu(C0 - Src1)
KAN_SPLINE_COMBINE = _register_op(
    "KAN_SPLINE_COMBINE",
    Spec(
        body=Src0 + sq(_r) * _r * C1,
        reference=lambda in0, in1, s0, s1, imm2: in0
        + s1 * np.maximum(s0 - in1, 0.0) ** 3,
    ),
)

# out = a^3 + s1 * relu(a - s0)^3   (in0 = a2 = relu(2-w); 1 stream)
_rb = relu(Src0 - C0)
KAN_TENT_POLY = _register_op(
    "KAN_TENT_POLY",
    Spec(
        body=sq(Src0) * Src0 + sq(_rb) * _rb * C1,
        reference=lambda in0, in1, s0, s1, imm2: in0**3
        + s1 * np.maximum(in0 - s0, 0.0) ** 3,
    ),
)

# a2 = relu(imm2 - |x*s0 + s1|)    (variant E pass 1; 1 stream, from x)
_u = Src0 * C0 + C1
_wabs = maxx(_u, Zero - _u)
KAN_A2_ABS = _register_op(
    "KAN_A2_ABS",
    Spec(
        body=relu(C2 - _wabs),
        reference=lambda in0, in1, s0, s1, imm2: np.maximum(
            imm2 - np.abs(in0 * s0 + s1), 0.0
        ),
    ),
)


def _chunk_groups(nic):
    """Basis-op batching: keep the first two chunks solo (short critical
    chain at layer boundaries), pair the rest."""
    groups = [[0]]
    if nic >= 2:
        groups.append([1])
    c = 2
    while c < nic:
        groups.append(list(range(c, min(c + 2, nic))))
        c += 2
    return groups


def _build_nc():
    nc = bacc.Bacc(trn_type="TRN2")
    xT_dr = nc.dram_tensor("xT", [WIDTH[0], BPC], F32, kind="ExternalInput")
    w_dr = [
        nc.dram_tensor(f"w{l}", [NB * WIDTH[l], WIDTH[l + 1]], BF16,
                       kind="ExternalInput")
        for l in range(4)
    ]
    out_dr = nc.dram_tensor("out", [BPC, WIDTH[4]], F32, kind="ExternalOutput")

    with tile.TileContext(nc) as tc:
        with (
            tc.tile_pool(name="const", bufs=1) as const_pool,
            tc.tile_pool(name="xt", bufs=2) as xt_pool,
            tc.tile_pool(name="ft", bufs=12) as ft_pool,
            tc.tile_pool(name="wt", bufs=8) as wt_pool,
            tc.tile_pool(name="tmp", bufs=4) as tmp_pool,
            tc.tile_pool(name="outp", bufs=1) as out_pool,
            tc.tile_pool(name="psum", bufs=8, space="PSUM") as psum_pool,
        ):
            # col g in 0..7: Abs bias 3.5-g ; col 8: +2.0 (variant-B Relu bias)
            bias = const_pool.tile([128, NB], F32, tag="bias")
            for g in range(NG):
                nc.gpsimd.memset(bias[:, g : g + 1], 3.5 - g)
            nc.gpsimd.memset(bias[:, NG : NG + 1], 2.0)
            ident = const_pool.tile([128, 128], F32, tag="ident")
            make_identity(nc, ident)

            nic0 = WIDTH[0] // 128
            xt = xt_pool.tile([128, nic0, BPC], F32, tag="xt")
            xT_r = xT_dr.rearrange("(c p) b -> p c b", p=128)
            # chunk 0 first, then the first weight tiles, then the rest
            nc.sync.dma_start(xt[:, 0:1, :], xT_r[:, 0:1, :])
            pre_wt = []
            for kb in range(3):
                wt = wt_pool.tile([128, WIDTH[1]], BF16, tag="wt",
                                  name=f"wt_pre_{kb}")
                nc.sync.dma_start(wt, w_dr[0][kb * 128 : (kb + 1) * 128, :])
                pre_wt.append(wt)
            for c in range(1, nic0):
                nc.sync.dma_start(xt[:, c : c + 1, :], xT_r[:, c : c + 1, :])

            def emit_fast_restart(l, src_psum):
                """First basis block of chunk 0 computed straight from the
                previous layer's PSUM so the PE restarts quickly."""
                a2 = tmp_pool.tile([128, BPC], F32, tag="qv",
                                   name=f"a2fr_{l}")
                nc.vector._custom_dve(KAN_A2_ABS, out=a2, in0=src_psum,
                                      s0=2.5, s1=3.5, imm2=2.0)
                ft0 = ft_pool.tile([128, NB, BPC], BF16, tag="ft",
                                   name=f"ft_{l}_0")
                nc.vector._custom_dve(KAN_TENT_POLY, out=ft0[:, 0, :],
                                      in0=a2, s0=1.0, s1=-4.0)
                return ft0

            def emit_copies(xt, src_psums, chunks):
                for i, c in enumerate(chunks):
                    if i % 2 == 0:
                        nc.scalar.copy(xt[:, c, :], src_psums[c])
                    else:
                        nc.vector.tensor_copy(xt[:, c, :], src_psums[c])

            def emit_basis(l, xt, c, ft, skip_g0=False):
                xa = xt[:, c, :]
                for g in range(NG):
                    if skip_g0 and g == 0:
                        continue
                    if g in VARIANT_B:
                        wv = tmp_pool.tile([128, BPC], F32, tag="wv",
                                           name=f"wv_{l}_{c}_{g}")
                        nc.scalar.activation(wv, xa, AF.Abs,
                                             bias=bias[:, g : g + 1],
                                             scale=2.5)
                        a2 = tmp_pool.tile([128, BPC], F32, tag="qv",
                                           name=f"a2_{l}_{c}_{g}")
                        nc.scalar.activation(a2, wv, AF.Relu,
                                             bias=bias[:, NG : NG + 1],
                                             scale=-1.0)
                    else:
                        a2 = tmp_pool.tile([128, BPC], F32, tag="qv",
                                           name=f"a2_{l}_{c}_{g}")
                        nc.vector._custom_dve(KAN_A2_ABS, out=a2, in0=xa,
                                              s0=2.5, s1=3.5 - g, imm2=2.0)
                    nc.vector._custom_dve(KAN_TENT_POLY, out=ft[:, g, :],
                                          in0=a2, s0=1.0, s1=-4.0)
                nc.scalar.activation(ft[:, NG, :], xa, AF.Silu)

            def emit_mms(l, c, ft, psums, ocs, col0, KB):
                dout = WIDTH[l + 1]
                ncol = len(ocs) * 128
                for g in range(NB):
                    kb = c * NB + g
                    if l == 0 and kb < len(pre_wt):
                        wt = pre_wt[kb]
                        wslice = lambda oc: wt[:, oc * 128 : (oc + 1) * 128]
                    else:
                        wt = wt_pool.tile([128, ncol], BF16, tag="wt",
                                          name=f"wt_{l}_{kb}_{col0}")
                        nc.sync.dma_start(
                            wt,
                            w_dr[l][kb * 128 : (kb + 1) * 128,
                                    col0 : col0 + ncol],
                        )
                        wslice = lambda oc: wt[:, (oc - ocs[0]) * 128 :
                                               (oc - ocs[0] + 1) * 128]
                    for oc in ocs:
                        nc.tensor.matmul(
                            psums[oc], wslice(oc), ft[:, g, :],
                            start=(kb == 0), stop=(kb == KB - 1),
                        )

            # ---- layers 0 and 1: plain per-chunk pipeline ----
            prev_psums = None
            for l in range(2):
                din, dout = WIDTH[l], WIDTH[l + 1]
                nic, noc = din // 128, dout // 128
                KB = NB * nic
                if prev_psums is not None:
                    xt = xt_pool.tile([128, nic, BPC], F32, tag="xt",
                                      name=f"xt_{l}")
                psums = [
                    psum_pool.tile([128, BPC], F32, tag="psum", name=f"ps_{l}_{i}")
                    for i in range(noc)
                ]
                for c in range(nic):
                    if c == 0 and prev_psums is not None:
                        ft = emit_fast_restart(l, prev_psums[0])
                        emit_copies(xt, prev_psums, range(nic))
                        emit_basis(l, xt, 0, ft, skip_g0=True)
                    else:
                        ft = ft_pool.tile([128, NB, BPC], BF16, tag="ft",
                                          name=f"ft_{l}_{c}")
                        emit_basis(l, xt, c, ft)
                    emit_mms(l, c, ft, psums, list(range(noc)), 0, KB)
                prev_psums = psums

            # ---- layer 2: out-chunk phase split (4 + 4 banks) so layer-3
            # basis production for early chunks overlaps phase B matmuls ----
            nic2, noc2 = WIDTH[2] // 128, WIDTH[3] // 128
            KB2 = NB * nic2
            xt2 = xt_pool.tile([128, nic2, BPC], F32, tag="xt", name="xt_2")
            psums2 = [
                psum_pool.tile([128, BPC], F32, tag="psum", name=f"ps_2_{i}")
                for i in range(noc2)
            ]
            l2_fts = []
            for c in range(nic2):
                if c == 0:
                    ft = emit_fast_restart(2, prev_psums[0])
                    emit_copies(xt2, prev_psums, range(nic2))
                    emit_basis(2, xt2, 0, ft, skip_g0=True)
                else:
                    ft = ft_pool.tile([128, NB, BPC], BF16, tag="ft",
                                      name=f"ft_2_{c}")
                    emit_basis(2, xt2, c, ft)
                l2_fts.append(ft)
                emit_mms(2, c, ft, psums2, [0, 1, 2, 3, 4], 0, KB2)

            # between phases: layer-3 input chunks 0..4 + their basis
            nic3, noc3 = WIDTH[3] // 128, WIDTH[4] // 128
            KB3 = NB * nic3
            xt3 = xt_pool.tile([128, nic3, BPC], F32, tag="xt", name="xt_3")
            ft3_0 = emit_fast_restart(3, psums2[0])
            emit_copies(xt3, psums2, range(5))
            psums3 = [
                psum_pool.tile([128, BPC], F32, tag="psum", name=f"ps_3_{i}")
                for i in range(noc3)
            ]
            l3_fts = [ft3_0]
            emit_basis(3, xt3, 0, ft3_0, skip_g0=True)
            for c in range(1, 5):
                ft = ft_pool.tile([128, NB, BPC], BF16, tag="ft",
                                  name=f"ft_3_{c}")
                emit_basis(3, xt3, c, ft)
                l3_fts.append(ft)

            # layer-2 phase B: out-chunks 5..7 (3-bank rotation keeps the
            # PE fill/drain overlap while finishing a bank-set early)
            for c in range(nic2):
                emit_mms(2, c, l2_fts[c], psums2, [5, 6, 7], 640, KB2)

            # layer-3 input chunks 5..7 + basis
            emit_copies(xt3, psums2, range(5, nic3))
            for c in range(5, nic3):
                ft = ft_pool.tile([128, NB, BPC], BF16, tag="ft",
                                  name=f"ft_3_{c}")
                emit_basis(3, xt3, c, ft)
                l3_fts.append(ft)

            # layer-3 matmuls
            for c in range(nic3):
                emit_mms(3, c, l3_fts[c], psums3, list(range(noc3)), 0, KB3)

            # output: transpose back to [batch, out]
            s3 = out_pool.tile([128, noc3, BPC], F32, tag="s3")
            for oc in range(noc3):
                nc.scalar.copy(s3[:, oc, :], psums3[oc])
            outT = out_pool.tile([128, BPC // 128, WIDTH[4]], F32, tag="outT")
            out_r = out_dr.rearrange("(j p) o -> p j o", p=128)
            for j in range(BPC // 128):
                for oc in range(noc3):
                    pst = psum_pool.tile([128, 128], F32, tag="psum",
                                         name=f"pst_{j}_{oc}")
                    nc.tensor.transpose(
                        pst, s3[:, oc, j * 128 : (j + 1) * 128], ident
                    )
                    nc.vector.tensor_copy(
                        outT[:, j, oc * 128 : (oc + 1) * 128], pst
                    )
                nc.sync.dma_start(
                    out_r[:, j : j + 1, :], outT[:, j : j + 1, :]
                )
    nc.finalize()
    return nc


_NC_CACHE = []


def _get_nc():
    if not _NC_CACHE:
        _NC_CACHE.append(_build_nc())
    return _NC_CACHE[0]


def _build_weights(inp):
    ws = {}
    for l in range(4):
        din, dout = WIDTH[l], WIDTH[l + 1]
        coef = np.asarray(inp[f"coef{l}"], dtype=np.float32)
        sb = np.asarray(inp[f"sb{l}"], dtype=np.float32)
        ss = np.asarray(inp[f"ss{l}"], dtype=np.float32)
        spline_w = coef * ss[:, :, None] * (1.0 / 6.0)  # [din, dout, 8]
        nic = din // 128
        sp = spline_w.reshape(nic, 128, dout, NG).transpose(0, 3, 1, 2)
        base = sb.reshape(nic, 128, dout)[:, None]
        W = np.concatenate([sp, base], axis=1).reshape(nic * NB * 128, dout)
        ws[f"w{l}"] = np.ascontiguousarray(W).astype(ml_dtypes.bfloat16)
    return ws


def _run(inputs, trace=False, **kwargs):
    inp = {k: np.asarray(v) for k, v in inputs.items()}
    ws = _build_weights(inp)
    x = np.concatenate(
        [inp["inputs_y"].astype(np.float32), inp["inputs_u"].astype(np.float32)],
        axis=1,
    )
    xT = np.ascontiguousarray(x.T)  # [512 feat, 4096 batch]
    nc = _get_nc()
    in_maps = []
    for c in range(NCORES):
        m = {"xT": np.ascontiguousarray(xT[:, c * BPC : (c + 1) * BPC])}
        m.update(ws)
        in_maps.append(m)
    res = run_bass_kernel_spmd(
        nc, in_maps, core_ids=list(range(NCORES)), trace=trace, **kwargs
    )
    out = np.concatenate([r["out"] for r in res.results], axis=0)
    return out.astype(np.float32), res


def kernel(**inputs) -> np.ndarray:
    out, _ = _run(inputs)
    return out


# revision 17
# speedup vs baseline: 1.3595x; 1.0163x over previous
"""KAN EncoderNetwork kernel for 8 Trainium2 NeuronCores.

Strategy (data-parallel, batch sharded 8 ways, weights replicated):

Each KAN layer  out = silu(x) @ sb + einsum('big,iog->bo', B(x), coef*ss)
is reformulated as ONE matmul per layer over an expanded feature matrix:

  out^T[o,b] = sum_K W'[K,o] * F[K,b]

where for every 128-wide input chunk the feature rows are 8 spline basis
blocks + 1 silu block (9*din rows total).  The uniform-grid cubic
B-spline basis has the closed form (cardinal spline, t = 2.5x + 5.5):

  6*B_g(x) = relu(2-w)^3 - 4*relu(1-w)^3,   w = |2.5x + 3.5 - g|

computed on ScalarE (Abs/Relu) + custom VectorE ops, balanced across the
two engines.  Everything stays feature-major ([feat, batch]) so layer
outputs in PSUM feed the next layer's basis computation directly; only
the final layer is transposed back (TensorE) for the [batch, out] output.

Weights are pre-assembled host-side into bf16 W' matrices with rows
ordered (in_chunk, block g in 0..8, lane) matching the feature layout.
"""

import sys

sys.path.insert(0, "/opt/trn_rl_repo")

import numpy as np
import ml_dtypes

import concourse.bacc as bacc
import concourse.mybir as mybir
import concourse.tile as tile
from concourse.bass_utils import run_bass_kernel_spmd
from concourse.masks import make_identity
from concourse.dve_spec import Spec, Src0, Src1, C0, C1, C2, Zero, relu, sq, maxx, lower, _has_src1
from concourse.dve_uop import DveOpSpec
from concourse.dve_ops import (
    DveOp,
    OPS,
    _SUB_OPCODE_FOR_NAME,
    CUSTOM_DVE_SPECS,
    _CUSTOM_DVE_ROW_BASE,
)

F32 = mybir.dt.float32
BF16 = mybir.dt.bfloat16
AF = mybir.ActivationFunctionType

WIDTH = [512, 1024, 1024, 1024, 256]
NCORES = 8
BATCH = 4096
BPC = BATCH // NCORES  # 512 batch rows per core
NG = 8  # spline basis functions per input dim
NB = NG + 1  # feature blocks per 128-chunk (8 basis + 1 silu)

# which basis functions use the ACT pipeline (B: Abs+Relu on ScalarE then
# one VectorE poly op) vs the all-DVE pipeline (E: two 1-stream VectorE ops)
VARIANT_B = {3, 4, 5, 6, 7}


def _register_op(name, spec):
    if name in _SUB_OPCODE_FOR_NAME:
        for op in OPS:
            if op.name == name:
                return op
        raise RuntimeError(f"opcode row taken but op {name} missing")
    row = _CUSTOM_DVE_ROW_BASE + len(OPS)
    _SUB_OPCODE_FOR_NAME[name] = row
    shas = {}
    for ver in ("v3", "v4"):
        uops = lower(spec, ver=ver)
        shas[ver] = DveOpSpec(
            name=name, opcode=row, uops=uops, rd1_en=_has_src1(spec)
        ).sha(ver)
    op = DveOp(name, spec, subdim=False, uops_sha=shas)
    OPS.append(op)
    CUSTOM_DVE_SPECS[name] = spec
    return op


# q = relu(s0 - w)^3        (variant A, pass 1; 1 stream)
_a = relu(C0 - Src0)
KAN_CUBE_TENT = _register_op(
    "KAN_CUBE_TENT",
    Spec(
        body=sq(_a) * _a,
        reference=lambda in0, in1, s0, s1, imm2: np.maximum(s0 - in0, 0.0) ** 3,
    ),
)

# out = q + s1 * relu(s0 - w)^3   (variant A, pass 2; in0=q, in1=w; 2 streams)
_r = relu(C0 - Src1)
KAN_SPLINE_COMBINE = _register_op(
    "KAN_SPLINE_COMBINE",
    Spec(
        body=Src0 + sq(_r) * _r * C1,
        reference=lambda in0, in1, s0, s1, imm2: in0
        + s1 * np.maximum(s0 - in1, 0.0) ** 3,
    ),
)

# out = a^3 + s1 * relu(a - s0)^3   (in0 = a2 = relu(2-w); 1 stream)
_rb = relu(Src0 - C0)
KAN_TENT_POLY = _register_op(
    "KAN_TENT_POLY",
    Spec(
        body=sq(Src0) * Src0 + sq(_rb) * _rb * C1,
        reference=lambda in0, in1, s0, s1, imm2: in0**3
        + s1 * np.maximum(in0 - s0, 0.0) ** 3,
    ),
)

# a2 = relu(imm2 - |x*s0 + s1|)    (variant E pass 1; 1 stream, from x)
_u = Src0 * C0 + C1
_wabs = maxx(_u, Zero - _u)
KAN_A2_ABS = _register_op(
    "KAN_A2_ABS",
    Spec(
        body=relu(C2 - _wabs),
        reference=lambda in0, in1, s0, s1, imm2: np.maximum(
            imm2 - np.abs(in0 * s0 + s1), 0.0
        ),
    ),
)


def _chunk_groups(nic):
    """Basis-op batching: keep the first two chunks solo (short critical
    chain at layer boundaries), pair the rest."""
    groups = [[0]]
    if nic >= 2:
        groups.append([1])
    c = 2
    while c < nic:
        groups.append(list(range(c, min(c + 2, nic))))
        c += 2
    return groups


def _build_nc():
    nc = bacc.Bacc(trn_type="TRN2")
    xT_dr = nc.dram_tensor("xT", [WIDTH[0], BPC], F32, kind="ExternalInput")
    w_dr = [
        nc.dram_tensor(f"w{l}", [NB * WIDTH[l], WIDTH[l + 1]], BF16,
                       kind="ExternalInput")
        for l in range(4)
    ]
    out_dr = nc.dram_tensor("out", [BPC, WIDTH[4]], F32, kind="ExternalOutput")

    with tile.TileContext(nc) as tc:
        with (
            tc.tile_pool(name="const", bufs=1) as const_pool,
            tc.tile_pool(name="xt", bufs=2) as xt_pool,
            tc.tile_pool(name="ft", bufs=12) as ft_pool,
            tc.tile_pool(name="wt", bufs=8) as wt_pool,
            tc.tile_pool(name="tmp", bufs=4) as tmp_pool,
            tc.tile_pool(name="outp", bufs=1) as out_pool,
            tc.tile_pool(name="psum", bufs=8, space="PSUM") as psum_pool,
        ):
            # col g in 0..7: Abs bias 3.5-g ; col 8: +2.0 (variant-B Relu bias)
            bias = const_pool.tile([128, NB], F32, tag="bias")
            for g in range(NG):
                nc.gpsimd.memset(bias[:, g : g + 1], 3.5 - g)
            nc.gpsimd.memset(bias[:, NG : NG + 1], 2.0)
            ident = const_pool.tile([128, 128], F32, tag="ident")
            make_identity(nc, ident)

            nic0 = WIDTH[0] // 128
            xt = xt_pool.tile([128, nic0, BPC], F32, tag="xt")
            xT_r = xT_dr.rearrange("(c p) b -> p c b", p=128)
            # chunk 0 first, then the first weight tiles, then the rest
            nc.sync.dma_start(xt[:, 0:1, :], xT_r[:, 0:1, :])
            pre_wt = []
            for kb in range(3):
                wt = wt_pool.tile([128, WIDTH[1]], BF16, tag="wt",
                                  name=f"wt_pre_{kb}")
                nc.sync.dma_start(wt, w_dr[0][kb * 128 : (kb + 1) * 128, :])
                pre_wt.append(wt)
            for c in range(1, nic0):
                nc.sync.dma_start(xt[:, c : c + 1, :], xT_r[:, c : c + 1, :])

            def emit_fast_restart(l, src_psum):
                """First basis block of chunk 0 computed straight from the
                previous layer's PSUM so the PE restarts quickly."""
                a2 = tmp_pool.tile([128, BPC], F32, tag="qv",
                                   name=f"a2fr_{l}")
                nc.vector._custom_dve(KAN_A2_ABS, out=a2, in0=src_psum,
                                      s0=2.5, s1=3.5, imm2=2.0)
                ft0 = ft_pool.tile([128, NB, BPC], BF16, tag="ft",
                                   name=f"ft_{l}_0")
                nc.vector._custom_dve(KAN_TENT_POLY, out=ft0[:, 0, :],
                                      in0=a2, s0=1.0, s1=-4.0)
                return ft0

            def emit_copies(xt, src_psums, chunks):
                for i, c in enumerate(chunks):
                    if i % 2 == 0:
                        nc.scalar.copy(xt[:, c, :], src_psums[c])
                    else:
                        nc.vector.tensor_copy(xt[:, c, :], src_psums[c])

            def emit_basis(l, xt, c, ft, skip_g0=False):
                xa = xt[:, c, :]
                for g in range(NG):
                    if skip_g0 and g == 0:
                        continue
                    if g in VARIANT_B:
                        wv = tmp_pool.tile([128, BPC], F32, tag="wv",
                                           name=f"wv_{l}_{c}_{g}")
                        nc.scalar.activation(wv, xa, AF.Abs,
                                             bias=bias[:, g : g + 1],
                                             scale=2.5)
                        a2 = tmp_pool.tile([128, BPC], F32, tag="qv",
                                           name=f"a2_{l}_{c}_{g}")
                        nc.scalar.activation(a2, wv, AF.Relu,
                                             bias=bias[:, NG : NG + 1],
                                             scale=-1.0)
                    else:
                        a2 = tmp_pool.tile([128, BPC], F32, tag="qv",
                                           name=f"a2_{l}_{c}_{g}")
                        nc.vector._custom_dve(KAN_A2_ABS, out=a2, in0=xa,
                                              s0=2.5, s1=3.5 - g, imm2=2.0)
                    nc.vector._custom_dve(KAN_TENT_POLY, out=ft[:, g, :],
                                          in0=a2, s0=1.0, s1=-4.0)
                nc.scalar.activation(ft[:, NG, :], xa, AF.Silu)

            def emit_mms(l, c, ft, psums, ocs, col0, KB):
                dout = WIDTH[l + 1]
                ncol = len(ocs) * 128
                for g in range(NB):
                    kb = c * NB + g
                    if l == 0 and kb < len(pre_wt):
                        wt = pre_wt[kb]
                        wslice = lambda oc: wt[:, oc * 128 : (oc + 1) * 128]
                    else:
                        wt = wt_pool.tile([128, ncol], BF16, tag="wt",
                                          name=f"wt_{l}_{kb}_{col0}")
                        nc.sync.dma_start(
                            wt,
                            w_dr[l][kb * 128 : (kb + 1) * 128,
                                    col0 : col0 + ncol],
                        )
                        wslice = lambda oc: wt[:, (oc - ocs[0]) * 128 :
                                               (oc - ocs[0] + 1) * 128]
                    for oc in ocs:
                        nc.tensor.matmul(
                            psums[oc], wslice(oc), ft[:, g, :],
                            start=(kb == 0), stop=(kb == KB - 1),
                        )

            # ---- layers 0 and 1: plain per-chunk pipeline ----
            prev_psums = None
            for l in range(2):
                din, dout = WIDTH[l], WIDTH[l + 1]
                nic, noc = din // 128, dout // 128
                KB = NB * nic
                if prev_psums is not None:
                    xt = xt_pool.tile([128, nic, BPC], F32, tag="xt",
                                      name=f"xt_{l}")
                psums = [
                    psum_pool.tile([128, BPC], F32, tag="psum", name=f"ps_{l}_{i}")
                    for i in range(noc)
                ]
                for c in range(nic):
                    if c == 0 and prev_psums is not None:
                        ft = emit_fast_restart(l, prev_psums[0])
                        emit_copies(xt, prev_psums, range(nic))
                        emit_basis(l, xt, 0, ft, skip_g0=True)
                    else:
                        ft = ft_pool.tile([128, NB, BPC], BF16, tag="ft",
                                          name=f"ft_{l}_{c}")
                        emit_basis(l, xt, c, ft)
                    emit_mms(l, c, ft, psums, list(range(noc)), 0, KB)
                prev_psums = psums

            # ---- layer 2: out-chunk phase split (4 + 4 banks) so layer-3
            # basis production for early chunks overlaps phase B matmuls ----
            nic2, noc2 = WIDTH[2] // 128, WIDTH[3] // 128
            KB2 = NB * nic2
            xt2 = xt_pool.tile([128, nic2, BPC], F32, tag="xt", name="xt_2")
            psums2 = [
                psum_pool.tile([128, BPC], F32, tag="psum", name=f"ps_2_{i}")
                for i in range(noc2)
            ]
            l2_fts = []
            for c in range(nic2):
                if c == 0:
                    ft = emit_fast_restart(2, prev_psums[0])
                    emit_copies(xt2, prev_psums, range(nic2))
                    emit_basis(2, xt2, 0, ft, skip_g0=True)
                else:
                    ft = ft_pool.tile([128, NB, BPC], BF16, tag="ft",
                                      name=f"ft_2_{c}")
                    emit_basis(2, xt2, c, ft)
                l2_fts.append(ft)
                emit_mms(2, c, ft, psums2, [0, 1, 2, 3], 0, KB2)

            # between phases: layer-3 input chunks 0..3 + their basis
            nic3, noc3 = WIDTH[3] // 128, WIDTH[4] // 128
            KB3 = NB * nic3
            xt3 = xt_pool.tile([128, nic3, BPC], F32, tag="xt", name="xt_3")
            ft3_0 = emit_fast_restart(3, psums2[0])
            emit_copies(xt3, psums2, range(4))
            psums3 = [
                psum_pool.tile([128, BPC], F32, tag="psum", name=f"ps_3_{i}")
                for i in range(noc3)
            ]
            l3_fts = [ft3_0]
            emit_basis(3, xt3, 0, ft3_0, skip_g0=True)
            for c in range(1, 4):
                ft = ft_pool.tile([128, NB, BPC], BF16, tag="ft",
                                  name=f"ft_3_{c}")
                emit_basis(3, xt3, c, ft)
                l3_fts.append(ft)

            # layer-2 phase B
            for c in range(nic2):
                emit_mms(2, c, l2_fts[c], psums2, [4, 5, 6, 7], 512, KB2)

            # layer-3 input chunks 4..7 + basis
            emit_copies(xt3, psums2, range(4, nic3))
            for c in range(4, nic3):
                ft = ft_pool.tile([128, NB, BPC], BF16, tag="ft",
                                  name=f"ft_3_{c}")
                emit_basis(3, xt3, c, ft)
                l3_fts.append(ft)

            # layer-3 matmuls
            for c in range(nic3):
                emit_mms(3, c, l3_fts[c], psums3, list(range(noc3)), 0, KB3)

            # output: transpose back to [batch, out]
            s3 = out_pool.tile([128, noc3, BPC], F32, tag="s3")
            for oc in range(noc3):
                nc.scalar.copy(s3[:, oc, :], psums3[oc])
            outT = out_pool.tile([128, BPC // 128, WIDTH[4]], F32, tag="outT")
            out_r = out_dr.rearrange("(j p) o -> p j o", p=128)
            for j in range(BPC // 128):
                for oc in range(noc3):
                    pst = psum_pool.tile([128, 128], F32, tag="psum",
                                         name=f"pst_{j}_{oc}")
                    nc.tensor.transpose(
                        pst, s3[:, oc, j * 128 : (j + 1) * 128], ident
                    )
                    nc.vector.tensor_copy(
                        outT[:, j, oc * 128 : (oc + 1) * 128], pst
                    )
                nc.sync.dma_start(
                    out_r[:, j : j + 1, :], outT[:, j : j + 1, :]
                )
    nc.finalize()
    return nc


_NC_CACHE = []


def _get_nc():
    if not _NC_CACHE:
        _NC_CACHE.append(_build_nc())
    return _NC_CACHE[0]


def _build_weights(inp):
    ws = {}
    for l in range(4):
        din, dout = WIDTH[l], WIDTH[l + 1]
        coef = np.asarray(inp[f"coef{l}"], dtype=np.float32)
        sb = np.asarray(inp[f"sb{l}"], dtype=np.float32)
        ss = np.asarray(inp[f"ss{l}"], dtype=np.float32)
        spline_w = coef * ss[:, :, None] * (1.0 / 6.0)  # [din, dout, 8]
        nic = din // 128
        sp = spline_w.reshape(nic, 128, dout, NG).transpose(0, 3, 1, 2)
        base = sb.reshape(nic, 128, dout)[:, None]
        W = np.concatenate([sp, base], axis=1).reshape(nic * NB * 128, dout)
        ws[f"w{l}"] = np.ascontiguousarray(W).astype(ml_dtypes.bfloat16)
    return ws


def _run(inputs, trace=False, **kwargs):
    inp = {k: np.asarray(v) for k, v in inputs.items()}
    ws = _build_weights(inp)
    x = np.concatenate(
        [inp["inputs_y"].astype(np.float32), inp["inputs_u"].astype(np.float32)],
        axis=1,
    )
    xT = np.ascontiguousarray(x.T)  # [512 feat, 4096 batch]
    nc = _get_nc()
    in_maps = []
    for c in range(NCORES):
        m = {"xT": np.ascontiguousarray(xT[:, c * BPC : (c + 1) * BPC])}
        m.update(ws)
        in_maps.append(m)
    res = run_bass_kernel_spmd(
        nc, in_maps, core_ids=list(range(NCORES)), trace=trace, **kwargs
    )
    out = np.concatenate([r["out"] for r in res.results], axis=0)
    return out.astype(np.float32), res


def kernel(**inputs) -> np.ndarray:
    out, _ = _run(inputs)
    return out
